# revision 1
# baseline (speedup 1.0000x reference)
"""Modulated conv2d (StyleGAN-2 style, B=16 C=128 HxW=128x128 K=3) on 8 TRN2
NeuronCores, data-parallel over batch (2 samples/core), ~150 us HW time.

Per core:
  1. style:  s[i,n] = (mod_w @ w_n) + mod_b + 1          (PE, K=512 via 4 k-tiles)
  2. wmod[i,t*C+o] = wT[i,t*C+o] * s[i]  -> bf16         (DVE per-partition scalar)
  3. dsq[o]  = sum_{i,t} wmod^2   via 9 accumulated matmuls with ones-vector rhs
  4. d[o]    = 1/sqrt(dsq + 1e-8)                        (ACT sqrt + DVE recip)
  5. conv:   x zero-padded to 130x130 on host, bf16; per 4-row output block,
     9 tap matmuls (K=C_in, M=C_out, N=512) accumulate fp32 in PSUM; the tap
     shift is a strided 3D rhs view into the padded image — no im2col.
  6. evict:  out = psum * d[o]                           (DVE tensor_scalar)
  7. DMA out (ACT-issued HWDGE), 12 staging buffers, 6 rotating PSUM banks.

The demod matmuls are interleaved into the first conv blocks so the
PE->ACT->DVE d-chain completes while PE streams; x arrives in 4 chunks with
a small first chunk so the conv can start early; all scalar params ride in
one packed [128, 1675] tensor (2 DMAs) to avoid serial small-DMA latency.

Raw Bass with manual semaphores: this toolchain's walrus accepts only ONE
sync-wait command per instruction, so Tile's auto-sync output does not
compile; explicit single-wait `wait_ge` instructions do. Every engine-pair
dependency (including same-engine RAW, which the hardware does not order)
is guarded by an explicit semaphore.

Numerics: bf16 operands, fp32 accumulation; max rel err vs the fp32 jax
reference ~2.3e-3. Set PRECISION = "f32r" for the float32r variant
(11-bit mantissa, rel err ~1.4e-4, ~10 us slower).
"""

import sys

sys.path.insert(0, "/opt/trn_rl_repo")

import numpy as np

import concourse.bass as bass
from concourse import mybir
from concourse.bass_utils import run_bass_kernel_spmd

B, C, H, W, KS, WD = 16, 128, 128, 128, 3, 512
NCORES = 8
SPC = B // NCORES          # samples per core = 2
HP = H + 2                 # padded height/width = 130
NT = KS * KS               # 9 taps
KT = WD // 128             # 4 k-tiles for the style matmul
PRECISION = "bf16"         # "bf16" (N=1024, FWL) or "f32r" (N=512, 11-bit mantissa)

R = 4                      # output rows per conv block (N = R*W = 512; PSUM bank cap)
NPS = 6                    # rotating conv PSUM banks
NOB = 12                   # output staging buffers
NB = H // R                # conv blocks per sample
CHUNK_BNDS = [0, 10, 50, 90, 130]   # x DMA chunk row boundaries (padded rows)


def _chunk_of_block(b):
    """First x chunk that covers padded rows needed by output block b."""
    need = R * b + R + 1
    for c in range(len(CHUNK_BNDS) - 1):
        if need < CHUNK_BNDS[c + 1]:
            return c
    raise AssertionError


F32 = mybir.dt.float32
F32R = mybir.dt.float32r
BF16 = mybir.dt.bfloat16
ADD = mybir.AluOpType.add
MULT = mybir.AluOpType.mult
SQRT = mybir.ActivationFunctionType.Sqrt


def round_fp32r(a):
    """Round fp32 array to fp32r (1s + 8e + 11m stored in top 20 bits, RNE)."""
    u = np.ascontiguousarray(a, np.float32).view(np.uint32)
    lower = u & np.uint32(0xFFF)
    keep_lsb = (u >> np.uint32(12)) & np.uint32(1)
    add = (lower > 0x800) | ((lower == 0x800) & (keep_lsb == 1))
    ru = (u & np.uint32(0xFFFFF000)) + (add.astype(np.uint32) << np.uint32(12))
    return ru.view(np.float32)


def build_program():
    nc = bass.Bass(trn_type="TRN2", target_bir_lowering=False, debug=False)
    xdt = BF16 if PRECISION == "bf16" else F32R

    NPS_R = KT * C + KT * SPC + 3            # f32r style: mwT | wvec | ones(x2) | modb
    NPF = 1 + NT * C                         # f32 params: eps | wT
    xpad_d = nc.dram_tensor("xpad", [SPC, C, HP, HP], xdt, kind="ExternalInput").ap()
    params_r_d = nc.dram_tensor("params_r", [C, NPS_R], F32R, kind="ExternalInput").ap()
    params_f_d = nc.dram_tensor("params_f", [C, NPF], F32, kind="ExternalInput").ap()
    y_d = nc.dram_tensor("y", [SPC, C, H, W], F32, kind="ExternalOutput").ap()

    xs = nc.alloc_sbuf_tensor("xs", [C, SPC, HP, HP], xdt).ap()
    params_r = nc.alloc_sbuf_tensor("params_r_sb", [C, NPS_R], F32R).ap()
    mwTs = params_r[:, 0 : KT * C].rearrange("p (k c) -> p k c", k=KT)
    wvecTs = params_r[:, KT * C : KT * C + KT * SPC].rearrange(
        "p (k c) -> p k c", k=KT)
    oness = params_r[:, NPS_R - 3 : NPS_R - 1]  # [C, 2] ones (fp32r MM needs even N)
    modbs = params_r[:, NPS_R - 1 : NPS_R].bitcast(F32)
    params_f = nc.alloc_sbuf_tensor("params_f_sb", [C, NPF], F32).ap()
    epss = params_f[:, 0:1]
    wTs = params_f[:, 1 : 1 + NT * C]
    wmod = nc.alloc_sbuf_tensor("wmod", [C, SPC, NT * C], xdt).ap()
    sq = nc.alloc_sbuf_tensor("sq", [C, SPC, NT * C], F32R).ap()
    outsb = nc.alloc_sbuf_tensor("outsb", [C, NOB, R * W], F32).ap()
    s_sb = nc.alloc_sbuf_tensor("s_sb", [C, SPC], F32).ap()
    dsr = nc.alloc_sbuf_tensor("dsr", [C, SPC], F32).ap()
    dcol = nc.alloc_sbuf_tensor("dcol", [C, SPC], F32).ap()

    cps = [nc.alloc_psum_tensor(f"cps{j}", [C, R * W], F32).ap() for j in range(NPS)]
    sps = nc.alloc_psum_tensor("sps", [C, SPC], F32).ap()
    dps = nc.alloc_psum_tensor("dps", [C, 2 * SPC], F32).ap()

    sem_x = [nc.alloc_semaphore(f"sx{i}") for i in range(SPC * 4)]
    sem_dma_param = nc.alloc_semaphore("sdma_param")   # style params (-> 16)
    sem_dma_wt = nc.alloc_semaphore("sdma_wt")         # wT (-> 16)
    sem_pe_style = nc.alloc_semaphore("pe_style")
    sem_dve_sq = nc.alloc_semaphore("dve_sq")
    sem_dve_w = nc.alloc_semaphore("dve_w")
    sem_pe_dcol = nc.alloc_semaphore("pe_dcol")
    sem_act_sqrt = nc.alloc_semaphore("act_sqrt")
    sem_pe_blk = nc.alloc_semaphore("pe_blk")
    sem_dve_evict = nc.alloc_semaphore("dve_evict")
    sem_dve_self = nc.alloc_semaphore("dve_self")
    sem_od = [nc.alloc_semaphore(f"sod{j}") for j in range(NOB)]

    with nc.Block() as blk:

        @blk.sync
        def _(eng):
            def xchunk(s, ci):
                r0, r1 = CHUNK_BNDS[ci], CHUNK_BNDS[ci + 1]
                eng.dma_start(
                    out=xs[:, s : s + 1, r0:r1, :],
                    in_=xpad_d[s : s + 1, :, r0:r1, :],
                ).then_inc(sem_x[4 * s + ci], 16)

            xchunk(0, 0)
            eng.dma_start(out=params_f, in_=params_f_d).then_inc(sem_dma_wt, 16)
            for ci in range(1, 4):
                xchunk(0, ci)
            for ci in range(4):
                xchunk(1, ci)

        @blk.tensor
        def _(eng):
            # style matmul: sps[i, n] = sum_d mod_w[i, d] * w[n, d]
            eng.wait_ge(sem_dma_param, 16)
            for kt in range(KT):
                inst = eng.matmul(
                    out=sps,
                    lhsT=mwTs[:, kt : kt + 1, :],
                    rhs=wvecTs[:, kt : kt + 1, :],
                    start=(kt == 0),
                    stop=(kt == KT - 1),
                )
            inst.then_inc(sem_pe_style, 1)

            def demod(s):
                # dps[o, s] = sum_{i, t} wmod[i, s, t*C+o]^2
                eng.wait_ge(sem_dve_sq, s + 1)
                if s >= 1:
                    eng.wait_ge(sem_act_sqrt, s)   # dps bank WAR vs ACT read
                for t in range(NT):
                    inst = eng.matmul(
                        out=dps[:, 2 * s : 2 * s + 2],
                        lhsT=sq[:, s : s + 1, t * C : (t + 1) * C],
                        rhs=oness,
                        start=(t == 0),
                        stop=(t == NT - 1),
                    )
                inst.then_inc(sem_pe_dcol, 1)

            def conv_block(s, b, gb):
                if b == 0 or _chunk_of_block(b) != _chunk_of_block(b - 1):
                    eng.wait_ge(sem_x[4 * s + _chunk_of_block(b)], 16)
                if gb >= NPS and (gb - NPS) % 4 == 0:
                    # covers bank reuse for blocks gb..gb+3 (reuse distance NPS)
                    eng.wait_ge(sem_dve_evict, gb - NPS + 4)
                for kh in range(KS):
                    for kw in range(KS):
                        t = kh * KS + kw
                        inst = eng.matmul(
                            out=cps[gb % NPS],
                            lhsT=wmod[:, s : s + 1, t * C : (t + 1) * C],
                            rhs=xs[:, s : s + 1, R * b + kh : R * b + kh + R,
                                   kw : kw + W],
                            start=(t == 0),
                            stop=(t == NT - 1),
                        )
                inst.then_inc(sem_pe_blk, 1)

            # interleave the demod matmuls into the first conv blocks so the
            # d-chain (PE->ACT->DVE) completes while PE streams early blocks
            eng.wait_ge(sem_dve_w, 1)
            conv_block(0, 0, 0)
            demod(0)
            conv_block(0, 1, 1)
            conv_block(0, 2, 2)
            demod(1)
            for b in range(3, NB):
                conv_block(0, b, b)
            eng.wait_ge(sem_dve_w, 2)
            for b in range(NB):
                conv_block(1, b, NB + b)

        @blk.vector
        def _(eng):
            eng.wait_ge(sem_pe_style, 1)
            eng.tensor_scalar(s_sb, sps, modbs, 1.0, ADD, ADD).then_inc(sem_dve_self, 1)
            eng.wait_ge(sem_dma_wt, 16)  # wT landed
            nself = 1
            for s in range(SPC):
                eng.wait_ge(sem_dve_self, nself)  # s_sb ready (same-engine RAW)
                eng.tensor_scalar(wmod[:, s : s + 1, :], wTs, s_sb[:, s : s + 1],
                                  None, MULT).then_inc(sem_dve_w, 1)
                eng.wait_ge(sem_dve_w, s + 1)     # wmod ready (same-engine RAW)
                wm_in = (wmod[:, s : s + 1, :] if PRECISION == "bf16"
                         else wmod[:, s : s + 1, :].bitcast(F32))
                eng.tensor_tensor(sq[:, s : s + 1, :], wm_in,
                                  wm_in, MULT).then_inc(sem_dve_sq, 1)
            for s in range(SPC):
                eng.wait_ge(sem_act_sqrt, s + 1)
                eng.reciprocal(dcol[:, s : s + 1], dsr[:, s : s + 1]).then_inc(
                    sem_dve_self, 1)
                nself += 1
            eng.wait_ge(sem_dve_self, nself)  # dcol ready for evictions
            # evictions: out = psum * d[o]
            for gb in range(SPC * NB):
                s = gb // NB
                eng.wait_ge(sem_pe_blk, gb + 1)
                if gb >= NOB:
                    eng.wait_ge(sem_od[gb % NOB], 16 * (gb // NOB))
                eng.tensor_scalar(outsb[:, gb % NOB : gb % NOB + 1, :],
                                  cps[gb % NPS], dcol[:, s : s + 1],
                                  None, MULT).then_inc(sem_dve_evict, 1)

        @blk.scalar
        def _(eng):
            # critical style-param DMA alone on ACT's HWDGE queue
            eng.dma_start(out=params_r, in_=params_r_d).then_inc(sem_dma_param, 16)
            for s in range(SPC):
                eng.wait_ge(sem_pe_dcol, s + 1)
                eng.activation(dsr[:, s : s + 1], dps[:, 2 * s : 2 * s + 1], SQRT,
                               bias=epss).then_inc(sem_act_sqrt, 1)
            # output DMAs (ACT is a HWDGE engine)
            for gb in range(SPC * NB):
                s, b = gb // NB, gb % NB
                eng.wait_ge(sem_dve_evict, gb + 1)
                eng.dma_start(
                    out=y_d[s : s + 1, :, R * b : R * b + R, :],
                    in_=outsb[:, gb % NOB : gb % NOB + 1, :],
                ).then_inc(sem_od[gb % NOB], 16)

    return nc


def _host_prep(x, w, weight, mod_w, mod_b):
    f = np.float32
    x = np.asarray(x, f)
    w = np.asarray(w, f)
    weight = np.asarray(weight, f)
    mod_w = np.asarray(mod_w, f)
    mod_b = np.asarray(mod_b, f)

    if PRECISION == "bf16":
        import ml_dtypes
        xpad = np.zeros((B, C, HP, HP), ml_dtypes.bfloat16)
        xpad[:, :, 1 : H + 1, 1 : W + 1] = x.astype(ml_dtypes.bfloat16)
    else:
        xpad = np.zeros((B, C, HP, HP), f)
        xpad[:, :, 1 : H + 1, 1 : W + 1] = round_fp32r(x)

    # params_r (f32r): mwT | wvecT | ones ; params_f (f32): modb | eps | wT
    NPS_R = KT * C + KT * SPC + 3
    NPF = 1 + NT * C
    # wT[i, t*C + o] = weight[o, i, kh, kw],  t = kh*3 + kw
    wT = weight.transpose(1, 2, 3, 0).reshape(C, NT * C)
    # mwT[d_lo, kt, i] = mod_w[i, kt*128 + d_lo]
    mwT = mod_w.T.reshape(KT, 128, C).transpose(1, 0, 2).reshape(C, KT * C)
    base_r = np.empty((C, NPS_R), f)
    base_r[:, : KT * C] = round_fp32r(mwT)
    base_r[:, NPS_R - 3 : NPS_R - 1] = 1.0
    base_r[:, NPS_R - 1] = round_fp32r(mod_b)
    base_f = np.empty((C, NPF), f)
    base_f[:, 0] = 1e-8
    base_f[:, 1:] = wT

    in_maps = []
    for core in range(NCORES):
        s0 = SPC * core
        # wvecT[d_lo, kt, n] = w[s0 + n, kt*128 + d_lo]
        wvecT = (w[s0 : s0 + SPC].T.reshape(KT, 128, SPC)
                 .transpose(1, 0, 2).reshape(C, KT * SPC))
        pr = base_r.copy()
        pr[:, KT * C : KT * C + KT * SPC] = round_fp32r(wvecT)
        in_maps.append({
            "xpad": np.ascontiguousarray(xpad[s0 : s0 + SPC]),
            "params_r": pr,
            "params_f": base_f,
        })
    return in_maps


_cached = {}


def kernel(x, w, weight, mod_w, mod_b):
    if "nc" not in _cached:
        _cached["nc"] = build_program()
    nc = _cached["nc"]
    in_maps = _host_prep(x, w, weight, mod_w, mod_b)
    res = run_bass_kernel_spmd(nc, in_maps, list(range(NCORES)))
    return np.concatenate([res.results[i]["y"] for i in range(NCORES)], axis=0)


if __name__ == "__main__":
    from concourse.bass_utils import compile_bass_kernel
    import tempfile

    nc = build_program()
    d = tempfile.mkdtemp()
    neff = compile_bass_kernel(nc, d)
    print("compiled OK:", neff)

